# revision 1
# baseline (speedup 1.0000x reference)
"""Deformable conv block (nn_DeformableConvolutionBlock) Trainium2 kernel.

Self-contained: builds a Bass/Tile program, runs it SPMD on 8 NeuronCores
(one batch sample per core), returns the full [8, 64, 256, 256] output.
"""

"""Split instructions carrying more than MAXW sem-waits into preceding
same-engine NOPs each carrying at most MAXW waits. Works around the
walrus TPB_CTRL 'Too many sync wait commands' codegen limit."""

from concourse import mybir

ENGINE_NS = {
    mybir.EngineType.PE: "tensor",
    mybir.EngineType.DVE: "vector",
    mybir.EngineType.Activation: "scalar",
    mybir.EngineType.Pool: "gpsimd",
    mybir.EngineType.SP: "sync",
}


def split_waits(nc, maxw: int = 2):
    for fn in nc.m.functions:
        for bb in fn.blocks:
            insts = list(bb.instructions)
            out = []
            changed = False
            for inst in insts:
                si = inst.sync_info
                if si is not None and len(si.on_wait) > maxw:
                    waits = list(si.on_wait)
                    # excess waits go on preceding nops, maxw at a time
                    excess, keep = waits[:-maxw], waits[-maxw:]
                    for i in range(0, len(excess), maxw):
                        eng = getattr(nc, ENGINE_NS[inst.engine])
                        nop = eng.nop(nofuse=True).ins
                        # remove the freshly-appended nop from wherever it landed
                        cur = nc.cur_bb.bb if nc.cur_bb is not None else None
                        for fb in nc.m.functions[0].blocks:
                            li = list(fb.instructions)
                            if li and li[-1].name == nop.name:
                                li.pop()
                                fb.instructions = li
                                break
                        nop.sync_info = mybir.SyncInfo(
                            on_wait=excess[i : i + maxw], on_update=[]
                        )
                        out.append(nop)
                    inst.sync_info = mybir.SyncInfo(
                        on_wait=keep, on_update=list(si.on_update)
                    )
                    changed = True
                out.append(inst)
            if changed:
                bb.instructions = out


"""Deformable conv block kernel for TRN2 (one sample per core).

Pipeline per core (input x: [C, H*W] f32, C=64):
  A) 1x1 convs on PE: y = w_conv @ x (kept), offs = [dy;dx;mlogit] per pixel
     (computed pixel-major via transposed matmuls); y transposed to y_t
     [pixel, ch] in DRAM via PE transposes.
  B) pointwise: bilinear corner indices + weights (incl. border zeroing),
     mask = sigmoid(mlogit + b_mask), all folded into 4 corner weights.
  C) indirect-DMA gather of corner pixel-pairs from y_t, weighted sum on
     DVE (pixel-major), PE transpose back to channel-major, DMA out.

Since the 1x1 conv is linear and per-pixel, it commutes with bilinear
sampling, so the conv runs FIRST and the gather samples y instead of x.
"""

from contextlib import ExitStack

import numpy as np

import concourse.bass as bass
import concourse.tile as tile
from concourse import mybir

F32 = mybir.dt.float32
I32 = mybir.dt.int32
AF = mybir.ActivationFunctionType
ALU = mybir.AluOpType

PAD = 128  # front/back padding rows of y_t (pair-base addressing can hit -1)


def build_nc(H, W, C=64, OC=64, CH_A=512, K=16, maxw=1, floor_mode="rne", debug=False):
    """Build the Bass program. CH_A: pixels per stage-A chunk (multiple of
    128, <=512). K: pixel-blocks (of 128) per stage-C gather chunk."""
    PIX = H * W
    NB = PIX // 128          # pixel blocks (lane l = p%128, block b = p//128)
    NCH_A = PIX // CH_A
    KA = CH_A // 128         # transpose blocks per A-chunk
    NCH_C = NB // K
    assert PIX % CH_A == 0 and NB % K == 0 and CH_A <= 512

    nc = bass.Bass()
    x_in = nc.declare_dram_parameter("x", [C, PIX], F32, isOutput=False)
    w3_in = nc.declare_dram_parameter("w3", [C, 3], F32, isOutput=False)
    wcT_in = nc.declare_dram_parameter("wcT", [C, OC], F32, isOutput=False)
    gyb_in = nc.declare_dram_parameter("gyb", [128, NB], F32, isOutput=False)
    gxb_in = nc.declare_dram_parameter("gxb", [128, NB], F32, isOutput=False)
    bm_in = nc.declare_dram_parameter("bm", [128, 1], F32, isOutput=False)
    id_in = nc.declare_dram_parameter("ident", [128, 128], F32, isOutput=False)
    out_ext = nc.declare_dram_parameter("out", [OC, PIX], F32, isOutput=True)

    y_t = nc.dram_tensor("y_t", [PIX + 2 * PAD, OC], F32)
    if debug:
        dbg_yt = nc.declare_dram_parameter("dbg_yt", [PIX + 2 * PAD, OC], F32, isOutput=True)
        dbg_r0 = nc.declare_dram_parameter("dbg_r0", [128, NB], I32, isOutput=True)
        dbg_r1 = nc.declare_dram_parameter("dbg_r1", [128, NB], I32, isOutput=True)
        dbg_wq = nc.declare_dram_parameter("dbg_wq", [128, NB, 4], F32, isOutput=True)
        dbg_off = nc.declare_dram_parameter("dbg_off", [128, NCH_A, KA, 3], F32, isOutput=True)
        dbg_g = nc.declare_dram_parameter("dbg_g", [128, 4, 64], F32, isOutput=True)
        dbg_s = nc.declare_dram_parameter("dbg_s", [128, 64], F32, isOutput=True)

    with TileCtx(nc) as (tc, ctx):
        const = ctx.enter_context(tc.tile_pool(name="const", bufs=1))
        persist = ctx.enter_context(tc.tile_pool(name="persist", bufs=1))

        w3_sb = const.tile([C, 3], F32)
        wcT_sb = const.tile([C, OC], F32)
        gyb_sb = const.tile([128, NB], F32)
        gxb_sb = const.tile([128, NB], F32)
        bm_sb = const.tile([128, 1], F32)
        id_sb = const.tile([128, 128], F32)
        nc.sync.dma_start(out=w3_sb[:], in_=w3_in[:])
        nc.sync.dma_start(out=wcT_sb[:], in_=wcT_in[:])
        nc.sync.dma_start(out=gyb_sb[:], in_=gyb_in[:])
        nc.sync.dma_start(out=gxb_sb[:], in_=gxb_in[:])
        nc.sync.dma_start(out=bm_sb[:], in_=bm_in[:])
        nc.sync.dma_start(out=id_sb[:], in_=id_in[:])

        off_store = persist.tile([128, NCH_A, KA, 3], F32)
        wq = persist.tile([128, NB, 4], F32)
        r0i = persist.tile([128, NB], I32)
        r1i = persist.tile([128, NB], I32)

        # ---------------- stage A ----------------
        with (
            tc.tile_pool(name="a_sbuf", bufs=3) as ap,
            tc.tile_pool(name="a_psum", bufs=2, space="PSUM") as app,
        ):
            zpad = ap.tile([128, OC], F32, tag="zpad")
            nc.vector.memset(zpad[:], 0.0)
            nc.sync.dma_start(out=y_t[0:PAD, :], in_=zpad[:])
            nc.sync.dma_start(out=y_t[PIX + PAD : PIX + 2 * PAD, :], in_=zpad[:])
            for ci in range(NCH_A):
                p0 = ci * CH_A
                x_t = ap.tile([C, CH_A], F32, tag="x")
                nc.sync.dma_start(out=x_t[:], in_=x_in[:, p0 : p0 + CH_A])

                ps_y = app.tile([OC, CH_A], F32, tag="ps_y")
                nc.tensor.matmul(out=ps_y[:], lhsT=wcT_sb[:], rhs=x_t[:],
                                 start=True, stop=True)
                y_sb = ap.tile([OC, CH_A], F32, tag="y")
                nc.scalar.copy(out=y_sb[:], in_=ps_y[:])

                ps_t = app.tile([128, KA * OC], F32, tag="ps_t")
                for k in range(KA):
                    nc.tensor.transpose(
                        out=ps_t[:, k * OC : (k + 1) * OC],
                        in_=y_sb[:, k * 128 : (k + 1) * 128],
                        identity=id_sb[:OC, :OC],
                    )
                yt_sb = ap.tile([128, KA * OC], F32, tag="yt")
                nc.scalar.copy(out=yt_sb[:], in_=ps_t[:])
                for k in range(KA):
                    nc.sync.dma_start(
                        out=y_t[PAD + p0 + k * 128 : PAD + p0 + (k + 1) * 128, :],
                        in_=yt_sb[:, k * OC : (k + 1) * OC],
                    )

                ps_o = app.tile([128, KA * 3], F32, tag="ps_o")
                for k in range(KA):
                    nc.tensor.matmul(
                        out=ps_o[:, k * 3 : (k + 1) * 3],
                        lhsT=x_t[:, k * 128 : (k + 1) * 128],
                        rhs=w3_sb[:],
                        start=True, stop=True,
                    )
                nc.vector.tensor_copy(out=off_store[:, ci, :, :], in_=ps_o[:])

        # ---------------- stage B ----------------
        with tc.tile_pool(name="b_sbuf", bufs=1) as bp:
            def t2(tag):
                return bp.tile([128, NB], F32, name=tag, tag=tag)

            po0, po1, pml = t2("po0"), t2("po1"), t2("pml")
            nc.vector.tensor_copy(out=po0[:].rearrange("p (a k) -> p a k", k=KA),
                                  in_=off_store[:, :, :, 0])
            nc.vector.tensor_copy(out=po1[:].rearrange("p (a k) -> p a k", k=KA),
                                  in_=off_store[:, :, :, 1])
            nc.vector.tensor_copy(out=pml[:].rearrange("p (a k) -> p a k", k=KA),
                                  in_=off_store[:, :, :, 2])

            def axis(po, gb, lim):
                """Returns (c0, v0, c1, v1, frac, base4) for one axis.
                po: offsets, gb: grid+bias+4 plane, lim: H-1 or W-1."""
                p4 = t2("p4")
                nc.vector.tensor_add(p4[:], po[:], gb[:])
                fr = t2("fr")
                nc.vector.tensor_scalar(out=fr[:], in0=p4[:], scalar1=1.0,
                                        scalar2=None, op0=ALU.mod)
                c04 = t2("c04")   # floor(p) + 4
                nc.vector.tensor_sub(c04[:], p4[:], fr[:])
                c0 = t2("c0")
                nc.vector.tensor_scalar(out=c0[:], in0=c04[:], scalar1=-4.0,
                                        scalar2=None, op0=ALU.add)
                c0c = t2("c0c")
                nc.vector.tensor_scalar(out=c0c[:], in0=c0[:], scalar1=0.0,
                                        scalar2=float(lim), op0=ALU.max, op1=ALU.min)
                v0 = t2("v0")
                nc.vector.tensor_tensor(out=v0[:], in0=c0[:], in1=c0c[:],
                                        op=ALU.is_equal)
                c1 = t2("c1")
                nc.vector.tensor_scalar(out=c1[:], in0=c0[:], scalar1=1.0,
                                        scalar2=None, op0=ALU.add)
                c1c = t2("c1c")
                nc.vector.tensor_scalar(out=c1c[:], in0=c1[:], scalar1=0.0,
                                        scalar2=float(lim), op0=ALU.max, op1=ALU.min)
                v1 = t2("v1")
                nc.vector.tensor_tensor(out=v1[:], in0=c1[:], in1=c1c[:],
                                        op=ALU.is_equal)
                w0 = t2("w0")   # (1-frac)*v0
                nc.vector.tensor_scalar(out=w0[:], in0=fr[:], scalar1=-1.0,
                                        scalar2=1.0, op0=ALU.mult, op1=ALU.add)
                nc.vector.tensor_mul(w0[:], w0[:], v0[:])
                w1 = t2("w1")   # frac*v1
                nc.vector.tensor_mul(w1[:], fr[:], v1[:])
                return c0c, c1c, w0, w1, c04

            # NOTE: axis() reuses tags -> second call would clobber the first
            # call's live tiles. Use distinct tag prefixes per axis instead.
            def axis_tagged(po, gb, lim, pre):
                def t(tag):
                    return bp.tile([128, NB], F32, name=pre + tag, tag=pre + tag)
                p4 = t("p4")
                nc.vector.tensor_add(p4[:], po[:], gb[:])
                fr = t("fr")
                c04 = t("c04")
                if floor_mode == "mod":
                    nc.vector.tensor_scalar(out=fr[:], in0=p4[:], scalar1=1.0,
                                            scalar2=None, op0=ALU.mod)
                    nc.vector.tensor_sub(c04[:], p4[:], fr[:])
                else:
                    # HW f32->i32 cast is round-nearest-even: floor(p) = rne(p-0.5)
                    # (at exact ints/halves the off-by-one corner gets weight 0/1
                    #  equivalently, so the sampled value is unchanged)
                    tm = t("tm")
                    nc.vector.tensor_scalar(out=tm[:], in0=p4[:], scalar1=-0.5,
                                            scalar2=None, op0=ALU.add)
                    ti = bp.tile([128, NB], I32, name=pre + "ti", tag=pre + "ti")
                    nc.vector.tensor_copy(out=ti[:], in_=tm[:])
                    nc.vector.tensor_copy(out=c04[:], in_=ti[:])
                    nc.vector.tensor_sub(fr[:], p4[:], c04[:])
                c0 = t("c0")
                nc.vector.tensor_scalar(out=c0[:], in0=c04[:], scalar1=-4.0,
                                        scalar2=None, op0=ALU.add)
                c0c = t("c0c")
                nc.vector.tensor_scalar(out=c0c[:], in0=c0[:], scalar1=0.0,
                                        scalar2=float(lim), op0=ALU.max, op1=ALU.min)
                v0 = t("v0")
                nc.vector.tensor_tensor(out=v0[:], in0=c0[:], in1=c0c[:],
                                        op=ALU.is_equal)
                c1 = t("c1")
                nc.vector.tensor_scalar(out=c1[:], in0=c0[:], scalar1=1.0,
                                        scalar2=None, op0=ALU.add)
                c1c = t("c1c")
                nc.vector.tensor_scalar(out=c1c[:], in0=c1[:], scalar1=0.0,
                                        scalar2=float(lim), op0=ALU.max, op1=ALU.min)
                v1 = t("v1")
                nc.vector.tensor_tensor(out=v1[:], in0=c1[:], in1=c1c[:],
                                        op=ALU.is_equal)
                w0 = t("w0")
                nc.vector.tensor_scalar(out=w0[:], in0=fr[:], scalar1=-1.0,
                                        scalar2=1.0, op0=ALU.mult, op1=ALU.add)
                nc.vector.tensor_mul(w0[:], w0[:], v0[:])
                w1 = t("w1")
                nc.vector.tensor_mul(w1[:], fr[:], v1[:])
                return c0c, c1c, w0, w1, c0

            y0c, y1c, wy0, wy1, _ = axis_tagged(po0, gyb_sb, H - 1, "y_")
            x0c, x1c, wx0, wx1, x0 = axis_tagged(po1, gxb_sb, W - 1, "x_")

            mask = bp.tile([128, NB], F32, tag="mask")
            nc.scalar.activation(out=mask[:], in_=pml[:], func=AF.Sigmoid,
                                 bias=bm_sb[:], scale=1.0)
            nc.vector.tensor_mul(wy0[:], wy0[:], mask[:])
            nc.vector.tensor_mul(wy1[:], wy1[:], mask[:])

            nc.vector.tensor_tensor(out=wq[:, :, 0], in0=wy0[:], in1=wx0[:],
                                    op=ALU.mult)
            nc.vector.tensor_tensor(out=wq[:, :, 1], in0=wy0[:], in1=wx1[:],
                                    op=ALU.mult)
            nc.vector.tensor_tensor(out=wq[:, :, 2], in0=wy1[:], in1=wx0[:],
                                    op=ALU.mult)
            nc.vector.tensor_tensor(out=wq[:, :, 3], in0=wy1[:], in1=wx1[:],
                                    op=ALU.mult)

            # pair-base column: clamp(x0, -1, W-1), then + PAD
            xb = bp.tile([128, NB], F32, tag="xb")
            nc.vector.tensor_scalar(out=xb[:], in0=x0[:], scalar1=-1.0,
                                    scalar2=float(W - 1), op0=ALU.max, op1=ALU.min)
            nc.vector.tensor_scalar(out=xb[:], in0=xb[:], scalar1=float(PAD),
                                    scalar2=None, op0=ALU.add)
            r0f = bp.tile([128, NB], F32, tag="r0f")
            nc.vector.scalar_tensor_tensor(out=r0f[:], in0=y0c[:],
                                           scalar=float(W), in1=xb[:],
                                           op0=ALU.mult, op1=ALU.add)
            r1f = bp.tile([128, NB], F32, tag="r1f")
            nc.vector.scalar_tensor_tensor(out=r1f[:], in0=y1c[:],
                                           scalar=float(W), in1=xb[:],
                                           op0=ALU.mult, op1=ALU.add)
            nc.vector.tensor_copy(out=r0i[:], in_=r0f[:])
            nc.vector.tensor_copy(out=r1i[:], in_=r1f[:])

        if debug:
            nc.sync.dma_start(out=dbg_r0[:], in_=r0i[:])
            nc.sync.dma_start(out=dbg_r1[:], in_=r1i[:])
            nc.sync.dma_start(out=dbg_wq[:], in_=wq[:])
            nc.sync.dma_start(out=dbg_off[:], in_=off_store[:])
            nc.sync.dma_start(out=dbg_yt[:], in_=y_t[:])

        # ---------------- stage C ----------------
        # One indirect call per 128-pixel block per corner-row (idx [128,1]
        # is the only offset form this walrus/firmware combo handles).
        NGRP = NB // K
        with (
            tc.tile_pool(name="c_sbuf", bufs=4) as cp,
            tc.tile_pool(name="c_psum", bufs=2, space="PSUM") as cpp,
        ):
            for cc in range(NGRP):
                ps = cpp.tile([OC, K * 128], F32, tag="ps")
                o_sb = cp.tile([OC, K * 128], F32, tag="o")
                for j in range(K):
                    b = cc * K + j
                    g = cp.tile([128, 4, OC], F32, tag="g")
                    gf = g[:].rearrange("p t c -> p (t c)")
                    nc.gpsimd.indirect_dma_start(
                        out=gf[:, 0 : 2 * OC], out_offset=None, in_=y_t[:],
                        in_offset=bass.IndirectOffsetOnAxis(
                            ap=r0i[:, b : b + 1], axis=0))
                    nc.gpsimd.indirect_dma_start(
                        out=gf[:, 2 * OC : 4 * OC], out_offset=None, in_=y_t[:],
                        in_offset=bass.IndirectOffsetOnAxis(
                            ap=r1i[:, b : b + 1], axis=0))
                    wv = bass.AP(tensor=wq.tensor, offset=wq.offset + b * 4,
                                 ap=[wq.ap[0], [1, 4], [0, OC]])
                    m = cp.tile([128, 4, OC], F32, tag="m")
                    nc.vector.tensor_tensor(out=m[:], in0=g[:], in1=wv,
                                            op=ALU.mult)
                    s = cp.tile([128, OC], F32, tag="s")
                    mt = bass.AP(tensor=m.tensor, offset=m.offset,
                                 ap=[m.ap[0], [1, OC], [OC, 4]])
                    nc.vector.reduce_sum(out=s[:], in_=mt,
                                         axis=mybir.AxisListType.X)
                    if debug and b == 0:
                        nc.sync.dma_start(out=dbg_g[:], in_=g[:])
                        nc.sync.dma_start(out=dbg_s[:], in_=s[:])
                    nc.tensor.transpose(
                        out=ps[:, j * 128 : (j + 1) * 128],
                        in_=s[:],
                        identity=id_sb[:],
                    )
                nc.scalar.copy(out=o_sb[:], in_=ps[:])
                nc.sync.dma_start(
                    out=out_ext[:, cc * K * 128 : (cc + 1) * K * 128],
                    in_=o_sb[:])

    split_waits(nc, maxw=maxw)
    return nc


class TileCtx:
    """TileContext + ExitStack in one `with`."""

    def __init__(self, nc):
        self.nc = nc

    def __enter__(self):
        self.ctx = ExitStack()
        self.ctx.__enter__()
        self.tc = self.ctx.enter_context(tile.TileContext(self.nc))
        return self.tc, self.ctx

    def __exit__(self, *a):
        return self.ctx.__exit__(*a)


def prep_core_inputs(xb, w_conv, w_off, b_off, w_mask, b_mask, H, W):
    C = xb.shape[0]
    PIX = H * W
    NB = PIX // 128
    p = np.arange(PIX, dtype=np.int64)
    lane = p % 128
    blk = p // 128
    gy = (p // W).astype(np.float32)
    gx = (p % W).astype(np.float32)
    gyb = np.zeros((128, NB), np.float32)
    gxb = np.zeros((128, NB), np.float32)
    gyb[lane, blk] = gy + np.float32(b_off[0]) + 4.0
    gxb[lane, blk] = gx + np.float32(b_off[1]) + 4.0
    return {
        "x": np.ascontiguousarray(xb.reshape(C, PIX)),
        "w3": np.ascontiguousarray(
            np.concatenate([w_off.T, w_mask.T], axis=1).astype(np.float32)),
        "wcT": np.ascontiguousarray(w_conv.T.astype(np.float32)),
        "gyb": gyb,
        "gxb": gxb,
        "bm": np.full((128, 1), np.float32(b_mask[0]), np.float32),
        "ident": np.eye(128, dtype=np.float32),
    }


def out_to_image(o, H, W):
    """[OC, PIX] with pixel index p (image row-major) -> [OC, H, W]."""
    return o.reshape(o.shape[0], H, W)


def np_reference(x, w_conv, w_off, b_off, w_mask, b_mask):
    B, C, Hh, Ww = x.shape
    off = np.einsum("bchw,oc->bohw", x, w_off) + b_off[None, :, None, None]
    ml = np.einsum("bchw,oc->bohw", x, w_mask) + b_mask[None, :, None, None]
    mask = 1.0 / (1.0 + np.exp(-ml))
    gy, gx = np.meshgrid(np.arange(Hh, dtype=x.dtype),
                         np.arange(Ww, dtype=x.dtype), indexing="ij")
    py = gy[None] + off[:, 0]
    px = gx[None] + off[:, 1]
    y0 = np.floor(py)
    x0 = np.floor(px)
    wy1 = py - y0
    wx1 = px - x0
    wy0 = 1.0 - wy1
    wx0 = 1.0 - wx1
    y0i = y0.astype(np.int64)
    x0i = x0.astype(np.int64)
    xf = x.reshape(B, C, Hh * Ww)

    def corner(yi, xi):
        valid = ((yi >= 0) & (yi < Hh) & (xi >= 0) & (xi < Ww)).astype(x.dtype)
        idx = np.clip(yi, 0, Hh - 1) * Ww + np.clip(xi, 0, Ww - 1)
        v = np.take_along_axis(xf, idx.reshape(B, 1, -1), axis=2)
        return v.reshape(B, C, Hh, Ww) * valid[:, None]

    s = (corner(y0i, x0i) * (wy0 * wx0)[:, None]
         + corner(y0i, x0i + 1) * (wy0 * wx1)[:, None]
         + corner(y0i + 1, x0i) * (wy1 * wx0)[:, None]
         + corner(y0i + 1, x0i + 1) * (wy1 * wx1)[:, None])
    s = s * mask
    return np.einsum("bchw,oc->bohw", s, w_conv)


# ----------------------------------------------------------------------
# host-side entry point
# ----------------------------------------------------------------------

_CACHE = {}

B_, C_, OC_, H_, W_ = 8, 64, 64, 256, 256


def _get_nc():
    if "nc" not in _CACHE:
        _CACHE["nc"] = build_nc(H_, W_, C_, OC_, CH_A=512, K=16)
    return _CACHE["nc"]


def _make_runner(nc, n_cores):
    """Persistent jitted SPMD runner (mirrors bass2jax.run_bass_via_pjrt,
    but reusable across calls so the NEFF/jit compile happens once)."""
    import jax
    from jax.sharding import Mesh, PartitionSpec
    from jax.experimental.shard_map import shard_map
    from concourse.bass2jax import (
        _bass_exec_p,
        install_neuronx_cc_hook,
        partition_id_tensor,
    )

    install_neuronx_cc_hook()
    partition_name = nc.partition_id_tensor.name if nc.partition_id_tensor else None
    in_names, out_names, out_avals, zero_outs = [], [], [], []
    for alloc in nc.m.functions[0].allocations:
        if not isinstance(alloc, mybir.MemoryLocationSet):
            continue
        name = alloc.memorylocations[0].name
        if alloc.kind == "ExternalInput":
            if name != partition_name:
                in_names.append(name)
        elif alloc.kind == "ExternalOutput":
            out_names.append(name)
            shape = tuple(alloc.tensor_shape)
            dtype = mybir.dt.np(alloc.dtype)
            out_avals.append(jax.core.ShapedArray(shape, dtype))
            zero_outs.append(np.zeros(shape, dtype))
    n_params = len(in_names)
    all_in = list(in_names) + list(out_names)
    if partition_name is not None:
        all_in.append(partition_name)

    def _body(*args):
        operands = list(args)
        if partition_name is not None:
            operands.append(partition_id_tensor())
        outs = _bass_exec_p.bind(
            *operands,
            out_avals=tuple(out_avals),
            in_names=tuple(all_in),
            out_names=tuple(out_names),
            lowering_input_output_aliases=(),
            sim_require_finite=True,
            sim_require_nnan=True,
            nc=nc,
        )
        return tuple(outs)

    devices = jax.devices()[:n_cores]
    mesh = Mesh(np.asarray(devices), ("core",))
    fn = jax.jit(
        shard_map(
            _body,
            mesh=mesh,
            in_specs=(PartitionSpec("core"),) * (n_params + len(out_names)),
            out_specs=(PartitionSpec("core"),) * len(out_names),
            check_rep=False,
        ),
        keep_unused=True,
    )
    return fn, in_names, out_names, zero_outs


def _get_runner():
    if "runner" not in _CACHE:
        _CACHE["runner"] = _make_runner(_get_nc(), B_)
    return _CACHE["runner"]


def kernel(x, w_conv, w_off, b_off, w_mask, b_mask):
    x = np.asarray(x, dtype=np.float32)
    w_conv = np.asarray(w_conv, dtype=np.float32)
    w_off = np.asarray(w_off, dtype=np.float32)
    b_off = np.asarray(b_off, dtype=np.float32)
    w_mask = np.asarray(w_mask, dtype=np.float32)
    b_mask = np.asarray(b_mask, dtype=np.float32)

    fn, in_names, out_names, zero_outs = _get_runner()
    per_core = [
        prep_core_inputs(x[b], w_conv, w_off, b_off, w_mask, b_mask, H_, W_)
        for b in range(B_)
    ]
    concat_in = [
        np.concatenate([per_core[c][n] for c in range(B_)], axis=0)
        for n in in_names
    ]
    concat_zero = [
        np.zeros((B_ * z.shape[0], *z.shape[1:]), z.dtype) for z in zero_outs
    ]
    outs = fn(*concat_in, *concat_zero)
    o = np.asarray(outs[out_names.index("out")]).reshape(B_, OC_, H_ * W_)
    return np.stack([out_to_image(o[b], H_, W_) for b in range(B_)], axis=0)


def _warmup():
    """Compile at import so the first kernel() call is fast."""
    try:
        z = {
            "x": np.zeros((B_, C_, H_, W_), np.float32),
            "w_conv": np.zeros((OC_, C_), np.float32),
            "w_off": np.zeros((2, C_), np.float32),
            "b_off": np.zeros((2,), np.float32),
            "w_mask": np.zeros((1, C_), np.float32),
            "b_mask": np.zeros((1,), np.float32),
        }
        kernel(**z)
    except Exception:
        _CACHE.clear()


_warmup()

